# revision 1
# baseline (speedup 1.0000x reference)
"""Trainium2 Bass kernel for ConvMultiHeadAttention (N=16, L=1024, E=512, H=8).

Data-parallel over batch: 8 NeuronCores x 2 batches each. Per core:
transposed-layout projections (fp16/fp32 mix), S^T = K_h^T-contract-d Q_h
attention logits with softmax-over-partitions via an appended ones column
in the AV matmul (denominator comes out as row 64 of the O accumulator),
reciprocals via exp(-ln d) on ScalarE, selector-matmul partition broadcast,
and a final fused output projection + bias.
"""

import numpy as np
import concourse.bass as bass
import concourse.mybir as mybir
import concourse.tile as tile
from contextlib import ExitStack
from concourse import bacc

P = 128
L = 1024
E = 512
H = 8
D = 64
NB = 2            # batches per core
TT = L // P       # 8 token tiles per batch
EPO = E // P      # 4 e-subtiles
FP32 = mybir.dt.float32
FP32R = mybir.dt.float16  # fp16 variant
BF16 = mybir.dt.float16
AF = mybir.ActivationFunctionType
ALU = mybir.AluOpType


def host_constants():
    import ml_dtypes
    ident = np.eye(P, dtype=np.float16)
    # sel2[p, 64h + j] = 1 iff p == 32*(h % 4): picks denom row of head h
    sel2 = np.zeros((P, H * D), np.float32)
    for h in range(H):
        sel2[32 * (h % 4), h * D:(h + 1) * D] = 1.0
    return ident, sel2


def build(debug=False):
    nc = bacc.Bacc("TRN2", target_bir_lowering=False, debug=debug)
    q_d = nc.dram_tensor("q", [NB, L, E], FP32, kind="ExternalInput").ap()
    k_d = nc.dram_tensor("k", [NB, L, E], FP32, kind="ExternalInput").ap()
    v_d = nc.dram_tensor("v", [NB, L, E], FP32, kind="ExternalInput").ap()
    wq_d = nc.dram_tensor("Wq", [E, E], FP32, kind="ExternalInput").ap()
    wk_d = nc.dram_tensor("Wk", [E, E], FP32, kind="ExternalInput").ap()
    wv_d = nc.dram_tensor("Wv", [E, E], FP32, kind="ExternalInput").ap()
    wo_d = nc.dram_tensor("Wo", [E, E], FP32, kind="ExternalInput").ap()
    bo_d = nc.dram_tensor("bo_bcast", [P, E], FP32, kind="ExternalInput").ap()
    id_d = nc.dram_tensor("ident", [P, P], BF16, kind="ExternalInput").ap()
    sel_d = nc.dram_tensor("sel2", [P, H * D], FP32, kind="ExternalInput").ap()
    out_d = nc.dram_tensor("out", [NB, L, E], FP32, kind="ExternalOutput").ap()

    with tile.TileContext(nc) as tc, ExitStack() as ctx:
        consts = ctx.enter_context(tc.tile_pool(name="consts", bufs=1))
        wt_pool = ctx.enter_context(tc.tile_pool(name="wt", bufs=1))
        xin_pool = ctx.enter_context(tc.tile_pool(name="xin", bufs=4))
        xt_pool = ctx.enter_context(tc.tile_pool(name="xt", bufs=3))
        qk_pool = ctx.enter_context(tc.tile_pool(name="qk", bufs=2))
        vh_pool = ctx.enter_context(tc.tile_pool(name="vh", bufs=2))
        st_pool = ctx.enter_context(tc.tile_pool(name="st", bufs=1))
        p_pool = ctx.enter_context(tc.tile_pool(name="pp", bufs=12))
        dn_pool = ctx.enter_context(tc.tile_pool(name="dn", bufs=1))
        o_pool = ctx.enter_context(tc.tile_pool(name="oo", bufs=3))
        ps_mm = ctx.enter_context(tc.tile_pool(name="psmm", bufs=2, space="PSUM"))
        ps_s = ctx.enter_context(tc.tile_pool(name="pss", bufs=2, space="PSUM"))
        ps_o = ctx.enter_context(tc.tile_pool(name="pso", bufs=2, space="PSUM"))

        # ---- constants ----
        ident = consts.tile([P, P], BF16)
        nc.sync.dma_start(ident[:], id_d)
        sel_f = xin_pool.tile([P, H * D], FP32, tag="xin")
        nc.sync.dma_start(sel_f[:], sel_d)
        sel = consts.tile([P, H * D], FP32R)
        nc.vector.tensor_copy(sel[:], sel_f[:])
        bo_t = consts.tile([P, E], FP32)
        nc.sync.dma_start(bo_t[:], bo_d)

        # ---- weight transposes: W [f, e] -> WT [e(pi), epo, f] fp32r ----
        wts = {}
        for wname, w_d in [("q", wq_d), ("k", wk_d), ("v", wv_d), ("o", wo_d)]:
            w_raw = xt_pool.tile([P, EPO, E], FP32, tag="xt")
            nc.sync.dma_start(w_raw[:], w_d.rearrange("(fo fi) e -> fi fo e", fi=P))
            w_nat = xt_pool.tile([P, EPO, E], BF16, tag="xtb")
            nc.vector.tensor_copy(w_nat[:], w_raw[:])
            wt = wt_pool.tile([P, EPO, E], FP32R, tag=f"wt_{wname}")
            for epo in range(EPO):
                ps = ps_mm.tile([P, E], BF16, tag="mm")
                for fpo in range(EPO):
                    nc.tensor.transpose(
                        ps[:, fpo * P:(fpo + 1) * P],
                        w_nat[:, fpo, epo * P:(epo + 1) * P],
                        ident[:],
                    )
                if wname == "q":
                    # fold 1/sqrt(D) into Wq
                    nc.vector.tensor_scalar_mul(wt[:, epo, :], ps[:], 1.0 / np.sqrt(D))
                else:
                    nc.vector.tensor_copy(wt[:, epo, :], ps[:])
            wts[wname] = wt

        out_tiles = []
        preps = {}
        for b in range(NB):
            # ======== prep: transposes + projections ========
            xts = {}
            for tname, x_d in [("q", q_d), ("k", k_d), ("v", v_d)]:
                xt = xt_pool.tile([P, EPO, L], FP32R, tag="xt")
                for tt in range(TT):
                    xin = xin_pool.tile([P, E], FP32, tag="xin")
                    nc.sync.dma_start(xin[:], x_d[b, tt * P:(tt + 1) * P, :])
                    xin_b = xin_pool.tile([P, E], BF16, tag="xinb")
                    nc.vector.tensor_copy(xin_b[:], xin[:])
                    ps = ps_mm.tile([P, E], BF16, tag="mm")
                    for epo in range(EPO):
                        nc.tensor.transpose(
                            ps[:, epo * P:(epo + 1) * P],
                            xin_b[:, epo * P:(epo + 1) * P],
                            ident[:],
                        )
                    # ps is [e-chunk x 4, t] blocks: block epo holds x^T[e(epo), t-tile]
                    nc.scalar.copy(
                        xt[:, :, tt * P:(tt + 1) * P],
                        ps[:].rearrange("p (epo t) -> p epo t", epo=EPO),
                    )
                xts[tname] = xt

            # qh^T, kh^T: [f(pi), fpo, t] = WT_x^T-contract-e @ x^T
            qkts = {}
            for tname in ["q", "k"]:
                wt = wts[tname]
                xt = xts[tname]
                ht = qk_pool.tile([P, EPO, L], FP32R, tag=f"ht_{tname}")
                for fpo in range(EPO):
                    for tch in range(L // E):  # 2 chunks of 512
                        ps = ps_mm.tile([P, E], FP32, tag="mm")
                        for epo in range(EPO):
                            nc.tensor.matmul(
                                ps[:],
                                wt[:, epo, fpo * P:(fpo + 1) * P],
                                xt[:, epo, tch * E:(tch + 1) * E],
                                start=(epo == 0),
                                stop=(epo == EPO - 1),
                            )
                        nc.vector.tensor_copy(ht[:, fpo, tch * E:(tch + 1) * E], ps[:])
                qkts[tname] = ht

            # vh natural [t(pi), tt, h, 65]; col 64 = ones
            vh = vh_pool.tile([P, TT, H, D + 1], FP32R, tag="vh")
            nc.vector.memset(vh[:], 1.0)  # ones col at [:,:,:,D]; rest overwritten
            wt = wts["v"]
            xt = xts["v"]
            for tt in range(TT):
                ps = ps_mm.tile([P, E], FP32, tag="mm")
                for epo in range(EPO):
                    nc.tensor.matmul(
                        ps[:],
                        xt[:, epo, tt * P:(tt + 1) * P],
                        wt[:, epo, :],
                        start=(epo == 0),
                        stop=(epo == EPO - 1),
                    )
                nc.vector.tensor_copy(
                    vh[:, tt, :, 0:D],
                    ps[:].rearrange("p (h d) -> p h d", h=H),
                )

            preps[b] = (qkts, vh)

        for b in range(NB):
            # ======== attention ========
            qkts, vh = preps[b]
            qht, kht = qkts["q"], qkts["k"]
            stage = st_pool.tile([P, EPO, L], FP32R, tag="st")
            denom = dn_pool.tile([P, 2, L], FP32, tag="dn")
            nc.vector.memset(denom[:], 1.0)
            def emit_s_exp(h):
                hpo, hoff = h // 2, D * (h % 2)
                pts = []
                for lt in range(TT):
                    pss = ps_s.tile([P, L], FP32, tag="s")
                    for ch in range(L // E):
                        nc.tensor.matmul(
                            pss[:, ch * E:(ch + 1) * E],
                            kht[hoff:hoff + D, hpo, lt * P:(lt + 1) * P],
                            qht[hoff:hoff + D, hpo, ch * E:(ch + 1) * E],
                            start=True,
                            stop=True,
                        )
                    pt = p_pool.tile([P, L], FP32R, tag="p")
                    nc.scalar.activation(pt[:], pss[:], AF.Exp)
                    pts.append(pt)
                return pts

            def emit_av(h, pts):
                hpo, hoff = h // 2, D * (h % 2)
                for ch in range(L // E):
                    pso = ps_o.tile([D + 1, E], FP32, tag="o")
                    for lt in range(TT):
                        nc.tensor.matmul(
                            pso[:],
                            vh[:, lt, h, :],
                            pts[lt][:, ch * E:(ch + 1) * E],
                            start=(lt == 0),
                            stop=(lt == TT - 1),
                        )
                    nc.vector.tensor_copy(
                        stage[hoff:hoff + D, hpo, ch * E:(ch + 1) * E], pso[0:D, :]
                    )
                    nc.vector.tensor_copy(
                        denom[32 * (h % 4):32 * (h % 4) + 1, h // 4, ch * E:(ch + 1) * E],
                        pso[D:D + 1, :],
                    )

            prev = None
            for h in range(H):
                pts = emit_s_exp(h)
                if prev is not None:
                    emit_av(prev[0], prev[1])
                prev = (h, pts)
            emit_av(prev[0], prev[1])

            # recip = exp(-ln(denom)) ; fp32r  (ln computed in place)
            nc.scalar.activation(denom[:], denom[:], AF.Ln)
            recip = dn_pool.tile([P, 2, L], FP32R, tag="dnr")
            nc.scalar.activation(recip[:], denom[:], AF.Exp, scale=-1.0)

            # normalize: stage[head] *= broadcast(recip[h])
            for h in range(H):
                hpo, hoff = h // 2, D * (h % 2)
                psb = ps_s.tile([D, L], FP32, tag="s")
                for ch in range(L // E):
                    nc.tensor.matmul(
                        psb[:, ch * E:(ch + 1) * E],
                        sel[:, h * D:(h + 1) * D],
                        recip[:, h // 4, ch * E:(ch + 1) * E],
                        start=True,
                        stop=True,
                    )
                nc.vector.tensor_tensor(
                    stage[hoff:hoff + D, hpo, :],
                    psb[:],
                    stage[hoff:hoff + D, hpo, :],
                    ALU.mult,
                )

            # ======== output projection ========
            wt = wts["o"]
            for tt in range(TT):
                ps = ps_mm.tile([P, E], FP32, tag="mm")
                for epo in range(EPO):
                    nc.tensor.matmul(
                        ps[:],
                        stage[:, epo, tt * P:(tt + 1) * P],
                        wt[:, epo, :],
                        start=(epo == 0),
                        stop=(epo == EPO - 1),
                    )
                ot = o_pool.tile([P, E], FP32, tag="ot")
                nc.vector.tensor_tensor(ot[:], ps[:], bo_t[:], ALU.add)
                nc.gpsimd.dma_start(out_d[b, tt * P:(tt + 1) * P, :], ot[:])
                out_tiles.append(ot)

    nc.compile()
    return nc




_COMPILED = None


def _get_compiled():
    global _COMPILED
    if _COMPILED is None:
        _COMPILED = build()
    return _COMPILED


def kernel(q, k, v, Wq, Wk, Wv, Wo, bo):
    import numpy as _np

    q = _np.ascontiguousarray(_np.asarray(q, dtype=_np.float32))
    k = _np.ascontiguousarray(_np.asarray(k, dtype=_np.float32))
    v = _np.ascontiguousarray(_np.asarray(v, dtype=_np.float32))
    Wq = _np.ascontiguousarray(_np.asarray(Wq, dtype=_np.float32))
    Wk = _np.ascontiguousarray(_np.asarray(Wk, dtype=_np.float32))
    Wv = _np.ascontiguousarray(_np.asarray(Wv, dtype=_np.float32))
    Wo = _np.ascontiguousarray(_np.asarray(Wo, dtype=_np.float32))
    bo = _np.asarray(bo, dtype=_np.float32)

    nc = _get_compiled()
    ident, sel2 = host_constants()
    bo_bcast = _np.ascontiguousarray(_np.broadcast_to(bo, (P, E)))
    n_cores = 8
    in_maps = []
    for c in range(n_cores):
        in_maps.append({
            "q": _np.ascontiguousarray(q[c * NB:(c + 1) * NB]),
            "k": _np.ascontiguousarray(k[c * NB:(c + 1) * NB]),
            "v": _np.ascontiguousarray(v[c * NB:(c + 1) * NB]),
            "Wq": Wq, "Wk": Wk, "Wv": Wv, "Wo": Wo,
            "bo_bcast": bo_bcast, "ident": ident, "sel2": sel2,
        })

    from concourse.bass_utils import run_bass_kernel_spmd
    res = run_bass_kernel_spmd(nc, in_maps, core_ids=list(range(n_cores)))
    out = _np.concatenate([res.results[c]["out"] for c in range(n_cores)], axis=0)
    return out.astype(_np.float32)



# revision 9
# speedup vs baseline: 1.3097x; 1.3097x over previous
"""Trainium2 Bass kernel for ConvMultiHeadAttention (N=16, L=1024, E=512, H=8).

Data-parallel over batch: 8 NeuronCores x 2 batches each. Host pre-transposes
and pre-casts inputs/weights (x^T layouts, fp8 with range pre-scaling), so the
device does only: fp8 DoubleRow projections (contract 256/matmul), row-tiled
fp16 QK^T logits (two heads run concurrently on disjoint PE row groups),
ScalarE exp into fp8 P tiles (bias folded, cancels in softmax ratio), fp8
DoubleRow AV with an appended ones column producing the softmax denominator in
row 64, reciprocal via exp(-ln d), selector-matmul partition broadcast of 1/d,
and a fused fp16 output projection + bias, streamed out via gpsimd-issued DMA.
"""

import numpy as np
import concourse.bass as bass
import concourse.mybir as mybir
import concourse.tile as tile
from contextlib import ExitStack
from collections import deque
from concourse import bacc

P = 128
L = 1024
E = 512
H = 8
D = 64
NB = 2            # batches per core
TT = L // P       # 8 token tiles
EPO = E // P      # 4 contract subtiles
HPAIR = H // 2    # 4 head pairs

FP8 = False
QSCALE = 64.0     # host pre-scale on Wq/sqrt(D) for fp8 dynamic range
WSCALE = 16.0     # host pre-scale on Wk/Wv

F32 = mybir.dt.float32
F16 = mybir.dt.float16
X8 = mybir.dt.float8e4 if FP8 else F16
AF = mybir.ActivationFunctionType
ALU = mybir.AluOpType
DRMODE = mybir.MatmulPerfMode.DoubleRow
EXP_BIAS = -1.5 if FP8 else 0.0
PT_BUFS = 16 if FP8 else 8


def build(debug=False):
    nc = bacc.Bacc("TRN2", target_bir_lowering=False, debug=debug)
    xds = {
        t: nc.dram_tensor(f"x{t}", [NB, P, EPO * L], X8, kind="ExternalInput").ap()
        for t in "qkv"
    }
    wds = {
        t: nc.dram_tensor(f"w{t}", [P, EPO * E], X8, kind="ExternalInput").ap()
        for t in "qkv"
    }
    wo_d = nc.dram_tensor("wo", [P, EPO * E], F16, kind="ExternalInput").ap()
    bo_d = nc.dram_tensor("bo", [P, E], F32, kind="ExternalInput").ap()
    sel_d = nc.dram_tensor("sel", [P, HPAIR * P], F16, kind="ExternalInput").ap()
    out_d = nc.dram_tensor("out", [NB, L, E], F32, kind="ExternalOutput").ap()

    with tile.TileContext(nc) as tc, ExitStack() as ctx:
        cpool = ctx.enter_context(tc.tile_pool(name="consts", bufs=1))
        big = ctx.enter_context(tc.tile_pool(name="big", bufs=1))
        ptp = ctx.enter_context(tc.tile_pool(name="ptp", bufs=16))
        otp = ctx.enter_context(tc.tile_pool(name="otp", bufs=4))
        psp = ctx.enter_context(tc.tile_pool(name="ps", bufs=1, space="PSUM"))

        # ---- constants ----
        bo_t = cpool.tile([P, E], F32, name="bo_t")
        nc.sync.dma_start(bo_t[:], bo_d)
        sel_t = cpool.tile([P, HPAIR * P], F16, name="sel_t")
        nc.sync.dma_start(sel_t[:], sel_d)
        expb = cpool.tile([P, 1], F32, name="expb")
        nc.vector.memset(expb[:], EXP_BIAS)
        wts = {}
        for t in "qkv":
            wt = big.tile([P, EPO, E], X8, tag=f"wt_{t}", name=f"wt_{t}")
            nc.sync.dma_start(wt[:], wds[t].rearrange("p (epo e) -> p epo e", epo=EPO))
            wts[t] = wt
        wo_t = big.tile([P, EPO, E], F16, tag="wt_o", name="wt_o")
        nc.sync.dma_start(wo_t[:], wo_d.rearrange("p (epo e) -> p epo e", epo=EPO))

        # ---- per-batch persistent tiles + input DMAs ----
        xts, hts, vhs, stages, denoms, recips = {}, {}, {}, {}, {}, {}
        for b in range(NB):
            for t in "qkv":
                xt = big.tile([P, EPO, L], X8, tag=f"xt_{t}{b}", name=f"xt_{t}{b}")
                nc.sync.dma_start(
                    xt[:], xds[t][b].rearrange("p (epo l) -> p epo l", epo=EPO)
                )
                xts[(t, b)] = xt
            for t in "qk":
                hts[(t, b)] = big.tile(
                    [P, HPAIR, L], F16, tag=f"ht_{t}{b}", name=f"ht_{t}{b}"
                )
            vhs[b] = big.tile([P, TT, H, D + 2], X8, tag=f"vh{b}", name=f"vh{b}")
            nc.vector.memset(vhs[b][:, :, :, D : D + 1], 1.0)
            stages[b] = big.tile([P, EPO, L], F16, tag=f"st{b}", name=f"st{b}")
            denoms[b] = big.tile([P, 2, L], F32, tag=f"dn{b}", name=f"dn{b}")
            nc.vector.memset(denoms[b][:], 1.0)
            recips[b] = big.tile([P, 2, L], F16, tag=f"rc{b}", name=f"rc{b}")

        def s_tile(nm):
            return psp.tile([P, L], F32, tag="s", bufs=3, name=nm)

        # ---- emission helpers ----
        def proj_qk_chunk(b, t, fpo):
            ps = s_tile(f"ps_{t}{b}{fpo}")
            wt, xt = wts[t], xts[(t, b)]
            for tch in range(2):
                o = ps[:, tch * E : (tch + 1) * E]
                if FP8:
                    for g in range(2):
                        nc.tensor.matmul(
                            o,
                            wt[:, 2 * g : 2 * g + 2, fpo * P : (fpo + 1) * P],
                            xt[:, 2 * g : 2 * g + 2, tch * E : (tch + 1) * E],
                            start=(g == 0),
                            stop=(g == 1),
                            perf_mode=DRMODE,
                        )
                else:
                    for epo in range(EPO):
                        nc.tensor.matmul(
                            o,
                            wt[:, epo, fpo * P : (fpo + 1) * P],
                            xt[:, epo, tch * E : (tch + 1) * E],
                            start=(epo == 0),
                            stop=(epo == EPO - 1),
                        )
            dst = hts[(t, b)][:, fpo, :]
            if FP8:
                sc = 1.0 / (QSCALE if t == "q" else WSCALE)
                nc.vector.tensor_scalar_mul(dst, ps[:], sc)
            else:
                nc.vector.tensor_copy(dst, ps[:])

        def vh_chunk(b, tt):
            ps = s_tile(f"pv_{b}{tt}")
            o = ps[:, 0:E]
            wt, xt = wts["v"], xts[("v", b)]
            if FP8:
                for g in range(2):
                    nc.tensor.matmul(
                        o,
                        xt[:, 2 * g : 2 * g + 2, tt * P : (tt + 1) * P],
                        wt[:, 2 * g : 2 * g + 2, :],
                        start=(g == 0),
                        stop=(g == 1),
                        perf_mode=DRMODE,
                    )
            else:
                for epo in range(EPO):
                    nc.tensor.matmul(
                        o,
                        xt[:, epo, tt * P : (tt + 1) * P],
                        wt[:, epo, :],
                        start=(epo == 0),
                        stop=(epo == EPO - 1),
                    )
            dst = vhs[b][:, tt, :, 0:D]
            src = o.rearrange("p (h d) -> p h d", h=H)
            if FP8:
                nc.vector.tensor_scalar_mul(dst, src, 1.0 / WSCALE)
            else:
                nc.vector.tensor_copy(dst, src)

        pts_store = {}

        def s_round(b, hp, lt):
            qht, kht = hts[("q", b)], hts[("k", b)]
            if lt % 2 == 0:
                pts_store.setdefault((b, hp, 0), []).append(
                    ptp.tile([P, 2, L], X8, tag="pt", bufs=PT_BUFS, name=f"ptA{b}{hp}{lt}")
                )
                pts_store.setdefault((b, hp, 1), []).append(
                    ptp.tile([P, 2, L], X8, tag="pt", bufs=PT_BUFS, name=f"ptB{b}{hp}{lt}")
                )
            ptA = pts_store[(b, hp, 0)][lt // 2]
            ptB = pts_store[(b, hp, 1)][lt // 2]
            psA = s_tile(f"sA{b}{hp}{lt}")
            psB = s_tile(f"sB{b}{hp}{lt}")
            for ch in range(2):
                nc.tensor.matmul(
                    psA[:, ch * E : (ch + 1) * E],
                    kht[0:64, hp, lt * P : (lt + 1) * P],
                    qht[0:64, hp, ch * E : (ch + 1) * E],
                    start=True,
                    stop=True,
                )
                nc.tensor.matmul(
                    psB[:, ch * E : (ch + 1) * E],
                    kht[64:128, hp, lt * P : (lt + 1) * P],
                    qht[64:128, hp, ch * E : (ch + 1) * E],
                    start=True,
                    stop=True,
                )
            nc.scalar.activation(ptA[:, lt % 2, :], psA[:], AF.Exp, bias=expb[:])
            nc.scalar.activation(ptB[:, lt % 2, :], psB[:], AF.Exp, bias=expb[:])

        pso_store = {}

        def av_group(b, hp, i, ch):
            h = 2 * hp + i
            vh = vhs[b]
            pts = pts_store[(b, hp, i)]
            if ch == 0:
                pso_store[(b, h)] = psp.tile(
                    [D + 1, L], F32, tag="o", bufs=1, name=f"pso{b}{h}"
                )
            pso = pso_store[(b, h)]
            o = pso[:, ch * E : (ch + 1) * E]
            if FP8:
                for tp in range(4):
                    nc.tensor.matmul(
                        o,
                        vh[:, 2 * tp : 2 * tp + 2, h, 0 : D + 1],
                        pts[tp][:, :, ch * E : (ch + 1) * E],
                        start=(tp == 0),
                        stop=(tp == 3),
                        perf_mode=DRMODE,
                    )
            else:
                for lt in range(TT):
                    nc.tensor.matmul(
                        o,
                        vh[:, lt, h, 0 : D + 1],
                        pts[lt // 2][:, lt % 2, ch * E : (ch + 1) * E],
                        start=(lt == 0),
                        stop=(lt == TT - 1),
                    )
            if ch == 1:
                hoff = 64 * (h % 2)
                nc.vector.tensor_copy(
                    stages[b][hoff : hoff + D, h // 2, :], pso[0:D, :]
                )
                nc.vector.tensor_copy(
                    denoms[b][32 * (h % 4) : 32 * (h % 4) + 1, h // 4, :],
                    pso[D : D + 1, :],
                )

        def norm(b):
            dn, rc, st = denoms[b], recips[b], stages[b]
            nc.scalar.activation(dn[:], dn[:], AF.Ln)
            nc.scalar.activation(rc[:], dn[:], AF.Exp, scale=-1.0)
            for hpo in range(HPAIR):
                psb = s_tile(f"psb{b}{hpo}")
                for ch in range(2):
                    nc.tensor.matmul(
                        psb[:, ch * E : (ch + 1) * E],
                        sel_t[:, hpo * P : (hpo + 1) * P],
                        rc[:, hpo // 2, ch * E : (ch + 1) * E],
                        start=True,
                        stop=True,
                    )
                nc.vector.tensor_tensor(
                    st[:, hpo, :], psb[:], st[:, hpo, :], ALU.mult
                )

        def outproj_chunk(b, tt):
            ps = s_tile(f"po{b}{tt}")
            o = ps[:, 0:E]
            st = stages[b]
            for epo in range(EPO):
                nc.tensor.matmul(
                    o,
                    st[:, epo, tt * P : (tt + 1) * P],
                    wo_t[:, epo, :],
                    start=(epo == 0),
                    stop=(epo == EPO - 1),
                )
            ot = otp.tile([P, E], F32, tag="ot", bufs=4, name=f"ot{b}{tt}")
            nc.vector.tensor_tensor(ot[:], o, bo_t[:], ALU.add)
            nc.gpsimd.dma_start(out_d[b, tt * P : (tt + 1) * P, :], ot[:])

        # ---- main emission ----
        # prologue: batch-0 projections
        for t in "qk":
            for fpo in range(HPAIR):
                proj_qk_chunk(0, t, fpo)
        for tt in range(TT):
            vh_chunk(0, tt)

        work = deque()
        for t in "qk":
            for fpo in range(HPAIR):
                work.append((proj_qk_chunk, (1, t, fpo)))
        for tt in range(TT):
            work.append((vh_chunk, (1, tt)))

        for b in range(NB):
            for hp in range(HPAIR):
                if hp > 0:
                    av_pending = deque(
                        (av_group, (b, hp - 1, i, ch))
                        for i in range(2)
                        for ch in range(2)
                    )
                elif b == 1:
                    av_pending = deque(
                        (av_group, (0, 3, i, ch)) for i in range(2) for ch in range(2)
                    )
                else:
                    av_pending = deque()
                for lt in range(TT):
                    s_round(b, hp, lt)
                    if av_pending and lt in (2, 3, 4, 5):
                        fn, args = av_pending.popleft()
                        fn(*args)
                    elif work and lt in (1, 2, 3, 4, 5, 6, 7):
                        fn, args = work.popleft()
                        fn(*args)
                while av_pending:
                    fn, args = av_pending.popleft()
                    fn(*args)
                if b == 1 and hp == 0:
                    norm(0)
                    for tt in range(TT):
                        work.append((outproj_chunk, (0, tt)))
        # tail: last pair's AV, norm, remaining outproj work, batch-1 outproj
        for i in range(2):
            for ch in range(2):
                av_group(1, 3, i, ch)
        while work:
            fn, args = work.popleft()
            fn(*args)
        norm(1)
        for tt in range(TT):
            outproj_chunk(1, tt)

    nc.compile()
    return nc


_COMPILED = None


def _get_compiled():
    global _COMPILED
    if _COMPILED is None:
        _COMPILED = build()
    return _COMPILED


def make_in_maps(q, k, v, Wq, Wk, Wv, Wo, bo):
    import ml_dtypes

    X8NP = ml_dtypes.float8_e4m3 if FP8 else np.float16
    N = q.shape[0]

    def xt_pack(x):
        xt = np.ascontiguousarray(np.asarray(x, np.float32).transpose(0, 2, 1))
        xt = xt.reshape(N, EPO, P, L).transpose(0, 2, 1, 3)
        return np.ascontiguousarray(xt.reshape(N, P, EPO * L).astype(X8NP))

    def wt_pack(W, scale, dt):
        wt = (np.asarray(W, np.float32).T * scale).reshape(EPO, P, E)
        wt = wt.transpose(1, 0, 2).reshape(P, EPO * E)
        return np.ascontiguousarray(wt.astype(dt))

    xq, xk, xv = xt_pack(q), xt_pack(k), xt_pack(v)
    qs = QSCALE if FP8 else 1.0
    ws = WSCALE if FP8 else 1.0
    wq = wt_pack(Wq, qs / np.sqrt(D), X8NP)
    wk = wt_pack(Wk, ws, X8NP)
    wv = wt_pack(Wv, ws, X8NP)
    wo = wt_pack(Wo, 1.0, np.float16)
    bo_b = np.ascontiguousarray(
        np.broadcast_to(np.asarray(bo, np.float32), (P, E))
    )
    sel = np.zeros((P, HPAIR * P), np.float16)
    for hpo in range(HPAIR):
        sel[32 * ((2 * hpo) % 4), hpo * P : hpo * P + 64] = 1
        sel[32 * ((2 * hpo + 1) % 4), hpo * P + 64 : (hpo + 1) * P] = 1

    in_maps = []
    for c in range(8):
        sl = slice(c * NB, (c + 1) * NB)
        in_maps.append({
            "xq": np.ascontiguousarray(xq[sl]),
            "xk": np.ascontiguousarray(xk[sl]),
            "xv": np.ascontiguousarray(xv[sl]),
            "wq": wq, "wk": wk, "wv": wv, "wo": wo,
            "bo": bo_b, "sel": sel,
        })
    return in_maps


def kernel(q, k, v, Wq, Wk, Wv, Wo, bo):
    nc = _get_compiled()
    in_maps = make_in_maps(q, k, v, Wq, Wk, Wv, Wo, bo)
    from concourse.bass_utils import run_bass_kernel_spmd

    res = run_bass_kernel_spmd(nc, in_maps, core_ids=list(range(8)))
    out = np.concatenate([res.results[c]["out"] for c in range(8)], axis=0)
    return out.astype(np.float32)
